# revision 3
# baseline (speedup 1.0000x reference)
"""Ring-attention (context-parallel) kernel for 8 TRN2 NeuronCores.

Problem: x_q [1,2048,2048], x_kv [1,8192,2048], GQA attention (16 q heads,
4 kv heads, D=128) where q occupies global positions 6144..8191 of the
8192-long key sequence (causal on the last 2048 block, full attention on
the first 6144 keys), followed by an output projection.

Strategy (sequence/context parallel, the module's native layout):
  - q rows are split into 16 strips of 128; core c owns strips {c, 15-c}
    (folded pairing -> every core attends to the same total number of keys,
    perfectly balancing the causal wedge).
  - x_kv is sequence-sharded 8 x 1024 rows; each core projects its local
    K/V shard to K^T / V (bf16); one AllGather PER HEAD GROUP shares the
    full K^T / V. Phase A is ordered so group g's bounce buffer completes
    as early as possible and AG_g fires immediately (V for all groups is
    pipelined against the input DMA on an 8-bank PSUM tile, then the four
    K_g projections follow, each followed by its collective trigger) ->
    the collectives overlap the Q projection and the first attention
    groups instead of serializing in front of them.
  - Projection weights are replicated (bf16).
  - Each core computes all 16 heads for its 256 q rows, then the full
    output projection for those rows -> no cross-core reduction at the end.

Engine budget: the attention inner loop is paced by the Scalar engine
(exp of the scores); all PSUM evacuations therefore go to the Vector
engine, and a third of the softmax-denominator accumulation is offloaded
to the otherwise-idle GpSimd engine.
"""

import numpy as np
import ml_dtypes

import concourse.bass as bass
import concourse.mybir as mybir
import concourse.tile as tile
from concourse import bacc, bass_utils

BF16 = ml_dtypes.bfloat16
F32 = mybir.dt.float32
BF = mybir.dt.bfloat16

N_CORES = 8
H = 16          # query heads
HKV = 4         # kv heads
D = 128         # head dim
HID = H * D     # 2048
SL = 2048       # q rows (global)
SKV = 8192      # kv rows (global)
QS = 256        # q rows per core (2 strips of 128)
LKV = SKV // N_CORES   # 1024 local kv rows
HC = HID // 128        # 16 hid chunks
KC = SKV // 128        # 64 key chunks
RANK_OFF = SKV - SL    # 6144: global position of q row 0
BND = RANK_OFF // 128  # 48: first key chunk needing a causal mask
SCALE = 1.0 / float(np.sqrt(D))

_CACHE = {}


def _build():
    nc = bacc.Bacc("TRN2", target_bir_lowering=False, debug=False,
                   num_devices=N_CORES)

    xqT = nc.dram_tensor("xqT", [HID, QS], BF, kind="ExternalInput")
    xkvT = nc.dram_tensor("xkvT", [HID, LKV], BF, kind="ExternalInput")
    wqT = nc.dram_tensor("wqT", [HID, HID], BF, kind="ExternalInput")
    wkT = nc.dram_tensor("wkT", [HID, HKV * D], BF, kind="ExternalInput")
    wvT = nc.dram_tensor("wvT", [HID, HKV * D], BF, kind="ExternalInput")
    woT = nc.dram_tensor("woT", [HID, HID], BF, kind="ExternalInput")
    # boundary causal masks: 16 key chunks (48..63) x [128 keys, 4 heads x 256 q]
    maskD = nc.dram_tensor("mask", [16 * 128, 4 * QS], BF, kind="ExternalInput")
    outT = nc.dram_tensor("outT", [HID, QS], F32, kind="ExternalOutput")

    with tile.TileContext(nc) as tc:
        _body(nc, tc, xqT, xkvT, wqT, wkT, wvT, woT, maskD, outT)
    nc.compile()
    return nc


def _body(nc, tc, xqT, xkvT, wqT, wkT, wvT, woT, maskD, outT):
    from contextlib import ExitStack
    ctx = ExitStack()
    with ctx:
        const = ctx.enter_context(tc.tile_pool(name="const", bufs=1))
        persist = ctx.enter_context(tc.tile_pool(name="persist", bufs=1))
        dram = ctx.enter_context(tc.tile_pool(name="dram", bufs=1, space="DRAM"))

        ones_k = const.tile([128, 1], BF)
        nc.gpsimd.memset(ones_k[:], 1.0)
        ones_1 = const.tile([1, 128], BF)
        nc.gpsimd.memset(ones_1[:], 1.0)

        qt_sb = persist.tile([128, H, QS], BF)        # Q^T per head
        ao_sb = persist.tile([128, HKV, 4 * QS], BF)  # normalized O^T per g

        # lower-priority resident loads on the gpsimd (SWDGE) queue so they
        # don't block the critical Phase A stream on the sync queue
        xq_sb = persist.tile([128, HC, QS], BF)   # Q^T input, hid-chunked
        nc.gpsimd.dma_start(
            xq_sb[:], xqT.ap().rearrange("(a p) q -> p a q", p=128))
        mask_sb = persist.tile([128, 16, 4 * QS], BF)
        nc.gpsimd.dma_start(
            mask_sb[:], maskD.ap().rearrange("(a p) q -> p a q", p=128))

        # ---------------- Phase A: local K/V projection -------------------
        # per-g AllGather bounce (1D): bytes [0 : 128*LKV) = K^T_g [D, LKV],
        # bytes [128*LKV : 256*LKV) = V_g [LKV, D] row-major
        bnc = [dram.tile([256 * LKV], BF, name=f"bnc{g}", uniquify=False)
               for g in range(HKV)]
        gath = [dram.tile([N_CORES * 256 * LKV], BF, addr_space="Shared",
                          name=f"gath{g}", uniquify=False)
                for g in range(HKV)]
        rg = [list(range(N_CORES))]

        with tc.tile_pool(name="kva", bufs=1) as kva:
            xkv_sb = kva.tile([128, HC, LKV], BF)
            wk_sb = kva.tile([128, HC, HKV * D], BF)
            wv_sb = kva.tile([128, HC, HKV * D], BF)
            for hc in range(HC):
                nc.sync.dma_start(
                    wv_sb[:, hc, :], wvT.ap()[hc * 128:(hc + 1) * 128, :])
                nc.sync.dma_start(
                    xkv_sb[:, hc, :], xkvT.ap()[hc * 128:(hc + 1) * 128, :])
                nc.sync.dma_start(
                    wk_sb[:, hc, :], wkT.ap()[hc * 128:(hc + 1) * 128, :])

            # V first, pipelined against the input DMA: all 8 key-chunk
            # accumulators live in one 8-bank PSUM tile, so every arriving
            # xkv chunk immediately feeds 8 matmuls; V is complete one
            # round after the last input chunk lands.
            with tc.tile_pool(name="vps", bufs=1, space="PSUM") as vps:
                ps_v = vps.tile([128, LKV // 128, HKV * D], F32)
                for hc in range(HC):
                    for lc in range(LKV // 128):
                        nc.tensor.matmul(
                            ps_v[:, lc, :],
                            xkv_sb[:, hc, lc * 128:(lc + 1) * 128],
                            wv_sb[:, hc, :],
                            start=(hc == 0), stop=(hc == HC - 1))
                for lc in range(LKV // 128):
                    v_loc = kva.tile([128, HKV * D], BF, tag="vloc")
                    nc.vector.tensor_copy(v_loc[:], ps_v[:, lc, :])
                    # scatter the 4 per-g column blocks into the per-g
                    # bounce tiles; V region is [LKV, D] row-major
                    for g in range(HKV):
                        nc.gpsimd.dma_start(
                            bnc[g][128 * LKV + lc * 128 * D:
                                   128 * LKV + (lc + 1) * 128 * D]
                            .rearrange("(p d) -> p d", p=128),
                            v_loc[:, g * D:(g + 1) * D])

            # K^T per group, each immediately followed by its AllGather:
            # AG_0 fires ~V_end + one K projection, and the 4 collectives
            # stream behind each other while Q projection / attention run.
            with tc.tile_pool(name="kps", bufs=2, space="PSUM") as kps:
                for g in range(HKV):
                    ps = kps.tile([128, LKV], F32, tag="kt")
                    for hc in range(HC):
                        lhsT = wk_sb[:, hc, g * D:(g + 1) * D]
                        for nn in range(0, LKV, 512):
                            nc.tensor.matmul(
                                ps[:, nn:nn + 512], lhsT,
                                xkv_sb[:, hc, nn:nn + 512],
                                start=(hc == 0), stop=(hc == HC - 1))
                    kt_loc = kva.tile([128, LKV], BF, tag="ktloc")
                    nc.vector.tensor_copy(kt_loc[:], ps[:])
                    nc.gpsimd.dma_start(
                        bnc[g][0:128 * LKV].rearrange("(p c) -> p c", p=128),
                        kt_loc[:])
                    nc.gpsimd.collective_compute(
                        "AllGather", mybir.AluOpType.bypass,
                        replica_groups=rg,
                        ins=[bnc[g].opt()], outs=[gath[g].opt()])

        # ---------------- Phase C: Q projection (overlaps AGs) ------------
        with (
            tc.tile_pool(name="qw", bufs=12) as qw,
            tc.tile_pool(name="qps", bufs=2, space="PSUM") as qps,
        ):
            for hb in range(4):            # blocks of 4 heads
                # one PSUM bank (512 f32) per head: interleaved accumulation
                # groups may not share a 2KB zero region
                ps = qps.tile([128, 4, 512], F32, tag="q")
                for hc in range(HC):
                    w_t = qw.tile([128, 512], BF, tag="wq")
                    nc.sync.dma_start(
                        w_t[:],
                        wqT.ap()[hc * 128:(hc + 1) * 128,
                                 hb * 512:(hb + 1) * 512])
                    for hh in range(4):
                        nc.tensor.matmul(
                            ps[:, hh, 0:QS],
                            w_t[:, hh * 128:(hh + 1) * 128],
                            xq_sb[:, hc, :],
                            start=(hc == 0), stop=(hc == HC - 1))
                for hh in range(4):
                    nc.vector.tensor_copy(
                        qt_sb[:, hb * 4 + hh, :], ps[:, hh, 0:QS])

        # ---------------- Phase D: attention ------------------------------
        with (
            tc.tile_pool(name="kvstream", bufs=3) as kvstream,
            tc.tile_pool(name="attw", bufs=3) as attw,
            tc.tile_pool(name="wop", bufs=2) as wop,
            tc.tile_pool(name="accp", bufs=1) as accp,
            tc.tile_pool(name="stps", bufs=2, space="PSUM") as stps,
            tc.tile_pool(name="otps", bufs=1, space="PSUM") as otps,
            tc.tile_pool(name="finps", bufs=1, space="PSUM") as finps,
            tc.tile_pool(name="fps", bufs=1, space="PSUM") as fps,
        ):
            out_acc = persist.tile([128, HC, QS], F32)
            W = 4 * QS            # 1024: 4 heads x 256 q
            for g in range(HKV):
                ot_ps = otps.tile([128, W], F32, tag="ot")
                # bf16 denominator accumulator, 2 chunk-halves wide (the two
                # halves are summed exactly in the f32 ones-matmul below)
                acc2 = accp.tile([128, 2 * W], BF, tag="acc")
                for r in range(N_CORES):
                    # stream rank r's K^T / V slabs for this head group
                    base = r * 256 * LKV
                    kt_slab = kvstream.tile([128, LKV], BF, tag="kt")
                    nc.sync.dma_start(
                        kt_slab[:],
                        gath[g][base:base + 128 * LKV]
                        .rearrange("(p c) -> p c", p=128))
                    v_slab = kvstream.tile([128, LKV // 128, D], BF, tag="v")
                    nc.sync.dma_start(
                        v_slab[:],
                        gath[g][base + 128 * LKV:base + 256 * LKV]
                        .rearrange("(a p d) -> p a d", p=128, d=D))
                    for l2 in range(LKV // 256):      # pairs of key chunks
                        ex = attw.tile([128, 2 * W], BF, tag="ex")
                        for eps in range(2):
                            l = l2 * 2 + eps
                            kc = r * (LKV // 128) + l
                            st = stps.tile([128, W], F32, tag="st")
                            exh = ex[:, eps * W:(eps + 1) * W]
                            if kc >= 56:
                                # strip-0 q cols are fully causal-masked for
                                # every core here: compute only the high half
                                # (the mask mul below zeroes the stale half)
                                for hh in range(4):
                                    nc.tensor.matmul(
                                        st[:, hh * QS + 128:(hh + 1) * QS],
                                        kt_slab[:, l * 128:(l + 1) * 128],
                                        qt_sb[:, g * 4 + hh, 128:QS],
                                        start=True, stop=True)
                                nc.scalar.activation(
                                    exh.rearrange(
                                        "p (h q) -> p h q", q=QS)[:, :, 128:],
                                    st[:].rearrange(
                                        "p (h q) -> p h q", q=QS)[:, :, 128:],
                                    mybir.ActivationFunctionType.Exp,
                                    scale=SCALE)
                            else:
                                for hh in range(0, 4, 2):
                                    nc.tensor.matmul(
                                        st[:, hh * QS:(hh + 2) * QS],
                                        kt_slab[:, l * 128:(l + 1) * 128],
                                        qt_sb[:, g * 4 + hh:g * 4 + hh + 2, :],
                                        start=True, stop=True)
                                nc.scalar.activation(
                                    exh, st[:],
                                    mybir.ActivationFunctionType.Exp,
                                    scale=SCALE)
                            if kc >= BND:
                                nc.vector.tensor_mul(
                                    exh, exh, mask_sb[:, kc - BND, :])
                            for nn in range(0, W, 512):
                                nc.tensor.matmul(
                                    ot_ps[:, nn:nn + 512],
                                    v_slab[:, l, :],
                                    ex[:, eps * W + nn:eps * W + nn + 512],
                                    start=(kc == 0), stop=(kc == KC - 1))
                        # denominator accumulation; every 3rd add goes to
                        # the otherwise-idle gpsimd engine so the vector
                        # engine keeps up with the scalar engine's exps
                        pi = r * (LKV // 256) + l2
                        if pi == 0:
                            nc.vector.tensor_copy(acc2[:], ex[:])
                        elif pi % 3 == 2:
                            nc.gpsimd.tensor_add(acc2[:], acc2[:], ex[:])
                        else:
                            nc.vector.tensor_add(acc2[:], acc2[:], ex[:])

                # copy attention output out unnormalized first, so ot_ps frees
                # for the next head group while the normalization tail runs
                nc.vector.tensor_copy(ao_sb[:, g, :], ot_ps[:])
                for nn in range(0, W, 512):
                    den = finps.tile([1, 512], F32, tag="fin")
                    nc.tensor.matmul(den[:], ones_k[:],
                                     acc2[:, nn:nn + 512],
                                     start=True, stop=False)
                    nc.tensor.matmul(den[:], ones_k[:],
                                     acc2[:, W + nn:W + nn + 512],
                                     start=False, stop=True)
                    recip_f = attw.tile([1, 512], F32, tag="recipf")
                    nc.vector.reciprocal(recip_f[:], den[:])
                    recip = attw.tile([1, 512], BF, tag="recip")
                    nc.vector.tensor_copy(recip[:], recip_f[:])
                    bc = finps.tile([128, 512], F32, tag="fin")
                    nc.tensor.matmul(bc[:], ones_1[:], recip[:],
                                     start=True, stop=True)
                    bc_sb = attw.tile([128, 512], BF, tag="bcsb")
                    nc.vector.tensor_copy(bc_sb[:], bc[:])
                    nc.vector.tensor_mul(ao_sb[:, g, nn:nn + 512],
                                         ao_sb[:, g, nn:nn + 512], bc_sb[:])

                # fold this head group into the output projection now (PE has
                # slack during the next group's attention); out_acc holds the
                # running f32 sum over groups
                wo_g = wop.tile([128, 4, HID], BF, tag="wog")
                nc.sync.dma_start(
                    wo_g[:],
                    woT.ap()[g * 512:(g + 1) * 512, :]
                    .rearrange("(a p) d -> p a d", p=128))
                for jc in range(HC):
                    fp = fps.tile([128, QS], F32, tag="fp")
                    for hh in range(4):
                        nc.tensor.matmul(
                            fp[:], wo_g[:, hh, jc * 128:(jc + 1) * 128],
                            ao_sb[:, g, hh * QS:(hh + 1) * QS],
                            start=(hh == 0), stop=(hh == 3))
                    if g == 0:
                        nc.vector.tensor_copy(out_acc[:, jc, :], fp[:])
                    else:
                        nc.vector.tensor_add(out_acc[:, jc, :],
                                             out_acc[:, jc, :], fp[:])

        # ---------------- Phase F: store the accumulated output -----------
        for jc in range(HC):
            nc.sync.dma_start(outT.ap()[jc * 128:(jc + 1) * 128, :],
                              out_acc[:, jc, :])


def _get_nc():
    if "nc" not in _CACHE:
        _CACHE["nc"] = _build()
    return _CACHE["nc"]


def _make_in_maps(x_q, x_kv, Wq, Wk, Wv, Wo):
    xqT_full = np.ascontiguousarray(x_q[0].T)           # [HID, SL]
    xkvT_full = np.ascontiguousarray(x_kv[0].T)         # [HID, SKV]
    wqT = np.ascontiguousarray(Wq.T).astype(BF16)
    wkT = np.ascontiguousarray(Wk.T).astype(BF16)
    wvT = np.ascontiguousarray(Wv.T).astype(BF16)
    woT = np.ascontiguousarray(Wo.T).astype(BF16)

    in_maps = []
    for c in range(N_CORES):
        s0, s1 = c, 15 - c
        xqT = np.concatenate(
            [xqT_full[:, s0 * 128:(s0 + 1) * 128],
             xqT_full[:, s1 * 128:(s1 + 1) * 128]], axis=1).astype(BF16)
        xkvT = np.ascontiguousarray(
            xkvT_full[:, c * LKV:(c + 1) * LKV]).astype(BF16)
        # causal masks for key chunks 48..63, replicated across the 4 heads
        # of a kv group (so one tensor_mul covers [128, 4*QS])
        mask = np.zeros((16, 128, QS), dtype=np.float32)
        kk = np.arange(128)
        for j in range(16):
            key_g = (BND + j) * 128 + kk                # [128]
            for half, st in enumerate((s0, s1)):
                q_g = RANK_OFF + st * 128 + np.arange(128)   # [128]
                mask[j, :, half * 128:(half + 1) * 128] = (
                    key_g[:, None] <= q_g[None, :])
        mask4 = np.tile(mask, (1, 1, 4))                # [16, 128, 4*QS]
        in_maps.append({
            "xqT": xqT, "xkvT": xkvT, "wqT": wqT, "wkT": wkT,
            "wvT": wvT, "woT": woT,
            "mask": mask4.reshape(16 * 128, 4 * QS).astype(BF16),
        })
    return in_maps


def _unshard(results):
    out = np.empty((1, SL, HID), dtype=np.float32)
    for c in range(N_CORES):
        outT = results[c]["outT"]                       # [HID, QS]
        s0, s1 = c, 15 - c
        out[0, s0 * 128:(s0 + 1) * 128, :] = outT[:, 0:128].T
        out[0, s1 * 128:(s1 + 1) * 128, :] = outT[:, 128:256].T
    return out


def kernel(x_q, x_kv, Wq, Wk, Wv, Wo, _trace=False, _result_box=None):
    nc = _get_nc()
    in_maps = _make_in_maps(x_q, x_kv, Wq, Wk, Wv, Wo)
    res = bass_utils.run_bass_kernel_spmd(
        nc, in_maps, core_ids=list(range(N_CORES)), trace=_trace)
    if _result_box is not None:
        _result_box.append(res)
    return _unshard(res.results)


# revision 4
# speedup vs baseline: 1.0647x; 1.0647x over previous
"""Ring-attention (context-parallel) kernel for 8 TRN2 NeuronCores.

Problem: x_q [1,2048,2048], x_kv [1,8192,2048], GQA attention (16 q heads,
4 kv heads, D=128) where q occupies global positions 6144..8191 of the
8192-long key sequence (causal on the last 2048 block, full attention on
the first 6144 keys), followed by an output projection.

Strategy (sequence/context parallel, the module's native layout):
  - q rows are split into 16 strips of 128; core c owns strips {c, 15-c}
    (folded pairing -> every core attends to the same total number of keys,
    perfectly balancing the causal wedge).
  - x_kv is sequence-sharded 8 x 1024 rows; each core projects its local
    K/V shard to K^T / V (bf16); per head group one AllGather for K^T and
    one for V share the full tensors. Phase A is ordered so group 0's
    gathers fire as early as possible (V for all groups pipelines against
    the input DMA on an 8-bank PSUM tile with independent evacuation
    buffers, then the four K_g projections follow, each chased by its two
    collective triggers) -> the collectives overlap the Q projection and
    the early attention groups instead of serializing in front of them.
  - Projection weights are replicated (bf16).
  - Each core computes all 16 heads for its 256 q rows, then the full
    output projection for those rows -> no cross-core reduction at the end.

Engine schedule: the attention inner loop is paced by the Scalar engine
(exp of the scores, ~1.2us per 128x1024 chunk), so the loop is software
pipelined: the PV matmuls of chunk i are emitted after the S matmuls of
chunk i+1, keeping the PE from head-of-line blocking on the exp. All PSUM
evacuations go to the Vector engine, and a third of the softmax
denominator accumulation goes to the otherwise-idle GpSimd engine.
"""

import numpy as np
import ml_dtypes

import concourse.bass as bass
import concourse.mybir as mybir
import concourse.tile as tile
from concourse import bacc, bass_utils

BF16 = ml_dtypes.bfloat16
F32 = mybir.dt.float32
BF = mybir.dt.bfloat16

N_CORES = 8
H = 16          # query heads
HKV = 4         # kv heads
D = 128         # head dim
HID = H * D     # 2048
SL = 2048       # q rows (global)
SKV = 8192      # kv rows (global)
QS = 256        # q rows per core (2 strips of 128)
LKV = SKV // N_CORES   # 1024 local kv rows
HC = HID // 128        # 16 hid chunks
KC = SKV // 128        # 64 key chunks
RANK_OFF = SKV - SL    # 6144: global position of q row 0
BND = RANK_OFF // 128  # 48: first key chunk needing a causal mask
SCALE = 1.0 / float(np.sqrt(D))

_CACHE = {}


def _build():
    nc = bacc.Bacc("TRN2", target_bir_lowering=False, debug=False,
                   num_devices=N_CORES)

    xqT = nc.dram_tensor("xqT", [HID, QS], BF, kind="ExternalInput")
    xkvT = nc.dram_tensor("xkvT", [HID, LKV], BF, kind="ExternalInput")
    wqT = nc.dram_tensor("wqT", [HID, HID], BF, kind="ExternalInput")
    wkT = nc.dram_tensor("wkT", [HID, HKV * D], BF, kind="ExternalInput")
    wvT = nc.dram_tensor("wvT", [HID, HKV * D], BF, kind="ExternalInput")
    woT = nc.dram_tensor("woT", [HID, HID], BF, kind="ExternalInput")
    # boundary causal masks: 16 key chunks (48..63) x [128 keys, 4 heads x 256 q]
    maskD = nc.dram_tensor("mask", [16 * 128, 4 * QS], BF, kind="ExternalInput")
    outT = nc.dram_tensor("outT", [HID, QS], F32, kind="ExternalOutput")

    with tile.TileContext(nc) as tc:
        _body(nc, tc, xqT, xkvT, wqT, wkT, wvT, woT, maskD, outT)
    nc.compile()
    return nc


def _body(nc, tc, xqT, xkvT, wqT, wkT, wvT, woT, maskD, outT):
    from contextlib import ExitStack
    ctx = ExitStack()
    with ctx:
        const = ctx.enter_context(tc.tile_pool(name="const", bufs=1))
        persist = ctx.enter_context(tc.tile_pool(name="persist", bufs=1))
        dram = ctx.enter_context(tc.tile_pool(name="dram", bufs=1, space="DRAM"))

        ones_k = const.tile([128, 1], BF)
        nc.gpsimd.memset(ones_k[:], 1.0)
        ones_1 = const.tile([1, 128], BF)
        nc.gpsimd.memset(ones_1[:], 1.0)
        # preload the ACT exp table during Phase A so the first attention
        # exp doesn't pay the ~2.7us ACT_TABLE_LOAD
        warm = const.tile([1, 2], F32)
        nc.scalar.activation(warm[:, 1:2], warm[:, 0:1],
                             mybir.ActivationFunctionType.Exp)

        qt_sb = persist.tile([128, H, QS], BF)        # Q^T per head
        ao_sb = persist.tile([128, HKV, 4 * QS], BF)  # normalized O^T per g

        # lower-priority resident loads on the gpsimd (SWDGE) queue so they
        # don't block the critical Phase A stream on the sync queue
        xq_sb = persist.tile([128, HC, QS], BF)   # Q^T input, hid-chunked
        nc.gpsimd.dma_start(
            xq_sb[:], xqT.ap().rearrange("(a p) q -> p a q", p=128))
        mask_sb = persist.tile([128, 16, 4 * QS], BF)
        nc.gpsimd.dma_start(
            mask_sb[:], maskD.ap().rearrange("(a p) q -> p a q", p=128))

        # ---------------- Phase A: local K/V projection -------------------
        # per-g bounce + gather tiles, K^T [D, LKV] and V [LKV, D] separate
        # so the K collective (which gates the first attention matmuls)
        # fires without waiting for V's transfer
        bncK = [dram.tile([128 * LKV], BF, name=f"bncK{g}", uniquify=False)
                for g in range(HKV)]
        bncV = [dram.tile([128 * LKV], BF, name=f"bncV{g}", uniquify=False)
                for g in range(HKV)]
        gathK = [dram.tile([N_CORES * 128 * LKV], BF, addr_space="Shared",
                           name=f"gathK{g}", uniquify=False)
                 for g in range(HKV)]
        gathV = [dram.tile([N_CORES * 128 * LKV], BF, addr_space="Shared",
                           name=f"gathV{g}", uniquify=False)
                 for g in range(HKV)]
        rg = [list(range(N_CORES))]

        with tc.tile_pool(name="kva", bufs=1) as kva:
            xkv_sb = kva.tile([128, HC, LKV], BF)
            wk_sb = kva.tile([128, HC, HKV * D], BF)
            wv_sb = kva.tile([128, HC, HKV * D], BF)
            for hc in range(HC):
                nc.sync.dma_start(
                    wv_sb[:, hc, :], wvT.ap()[hc * 128:(hc + 1) * 128, :])
                nc.sync.dma_start(
                    xkv_sb[:, hc, :], xkvT.ap()[hc * 128:(hc + 1) * 128, :])
                nc.sync.dma_start(
                    wk_sb[:, hc, :], wkT.ap()[hc * 128:(hc + 1) * 128, :])

            # V first, pipelined against the input DMA: all 8 key-chunk
            # accumulators live in one 8-bank PSUM tile, so every arriving
            # xkv chunk immediately feeds 8 matmuls; V is complete one
            # round after the last input chunk lands. v_all gives every
            # chunk an independent evacuation slice (no WAR chaining).
            v_all = kva.tile([128, LKV // 128, HKV * D], BF)
            with tc.tile_pool(name="vps", bufs=1, space="PSUM") as vps:
                ps_v = vps.tile([128, LKV // 128, HKV * D], F32)
                for hc in range(HC):
                    for lc in range(LKV // 128):
                        nc.tensor.matmul(
                            ps_v[:, lc, :],
                            xkv_sb[:, hc, lc * 128:(lc + 1) * 128],
                            wv_sb[:, hc, :],
                            start=(hc == 0), stop=(hc == HC - 1))
                for lc in range(LKV // 128):
                    nc.vector.tensor_copy(v_all[:, lc, :], ps_v[:, lc, :])
            # one bounce write per group; V region is [LKV, D] row-major
            for g in range(HKV):
                nc.gpsimd.dma_start(
                    bncV[g].rearrange("(a p d) -> p a d", p=128, d=D),
                    v_all[:, :, g * D:(g + 1) * D])

            # K^T per group, each immediately followed by its AllGathers:
            # AG_0K fires ~V_end + one K projection, and the 8 collectives
            # stream behind each other while Q projection / attention run.
            with tc.tile_pool(name="kps", bufs=2, space="PSUM") as kps:
                for g in range(HKV):
                    ps = kps.tile([128, LKV], F32, tag="kt")
                    for hc in range(HC):
                        lhsT = wk_sb[:, hc, g * D:(g + 1) * D]
                        for nn in range(0, LKV, 512):
                            nc.tensor.matmul(
                                ps[:, nn:nn + 512], lhsT,
                                xkv_sb[:, hc, nn:nn + 512],
                                start=(hc == 0), stop=(hc == HC - 1))
                    kt_loc = kva.tile([128, LKV], BF, tag=f"ktloc{g}")
                    nc.vector.tensor_copy(kt_loc[:], ps[:])
                    nc.gpsimd.dma_start(
                        bncK[g].rearrange("(p c) -> p c", p=128), kt_loc[:])
                    nc.gpsimd.collective_compute(
                        "AllGather", mybir.AluOpType.bypass,
                        replica_groups=rg,
                        ins=[bncK[g].opt()], outs=[gathK[g].opt()])
                    nc.gpsimd.collective_compute(
                        "AllGather", mybir.AluOpType.bypass,
                        replica_groups=rg,
                        ins=[bncV[g].opt()], outs=[gathV[g].opt()])

        # ---------------- Phase C: Q projection (overlaps AGs) ------------
        with (
            tc.tile_pool(name="qw", bufs=12) as qw,
            tc.tile_pool(name="qps", bufs=2, space="PSUM") as qps,
        ):
            for hb in range(4):            # blocks of 4 heads
                # one PSUM bank (512 f32) per head: interleaved accumulation
                # groups may not share a 2KB zero region
                ps = qps.tile([128, 4, 512], F32, tag="q")
                for hc in range(HC):
                    w_t = qw.tile([128, 512], BF, tag="wq")
                    nc.sync.dma_start(
                        w_t[:],
                        wqT.ap()[hc * 128:(hc + 1) * 128,
                                 hb * 512:(hb + 1) * 512])
                    for hh in range(4):
                        nc.tensor.matmul(
                            ps[:, hh, 0:QS],
                            w_t[:, hh * 128:(hh + 1) * 128],
                            xq_sb[:, hc, :],
                            start=(hc == 0), stop=(hc == HC - 1))
                for hh in range(4):
                    nc.vector.tensor_copy(
                        qt_sb[:, hb * 4 + hh, :], ps[:, hh, 0:QS])

        # ---------------- Phase D: attention ------------------------------
        with (
            tc.tile_pool(name="kvstream", bufs=3) as kvstream,
            tc.tile_pool(name="attw", bufs=4) as attw,
            tc.tile_pool(name="wop", bufs=2) as wop,
            tc.tile_pool(name="accp", bufs=1) as accp,
            tc.tile_pool(name="stps", bufs=2, space="PSUM") as stps,
            tc.tile_pool(name="otps", bufs=1, space="PSUM") as otps,
            tc.tile_pool(name="finps", bufs=1, space="PSUM") as finps,
            tc.tile_pool(name="fps", bufs=1, space="PSUM") as fps,
        ):
            out_acc = persist.tile([128, HC, QS], F32)
            W = 4 * QS            # 1024: 4 heads x 256 q
            for g in range(HKV):
                ot_ps = otps.tile([128, W], F32, tag="ot")
                # bf16 denominator accumulator, 2 chunk-halves wide (the two
                # halves are summed exactly in the f32 ones-matmul below)
                acc2 = accp.tile([128, 2 * W], BF, tag="acc")

                # ---- software-pipelined chunk loop: S+exp for chunk kc is
                # emitted first, then PV + denominator-add for chunk kc-1,
                # so the PE never waits on the exp of the current chunk.
                pend = []      # [(ex_tile, l_in_slab, v_slab, kc)]

                def flush(pend=pend):
                    ex, l, v_slab, kc = pend.pop(0)
                    if kc >= 56:
                        for hh in range(4):
                            nc.tensor.matmul(
                                ot_ps[:, hh * QS + 128:(hh + 1) * QS],
                                v_slab[:, l, :],
                                ex[:, hh * QS + 128:(hh + 1) * QS],
                                start=(kc == 0), stop=(kc == KC - 1))
                    else:
                        for nn in range(0, W, 512):
                            nc.tensor.matmul(
                                ot_ps[:, nn:nn + 512],
                                v_slab[:, l, :],
                                ex[:, nn:nn + 512],
                                start=(kc == 0), stop=(kc == KC - 1))
                    # denominator accumulation; every 3rd add runs on the
                    # otherwise-idle gpsimd engine so the vector engine
                    # keeps pace with the scalar engine's exps
                    half = (kc % 2) * W
                    dst = acc2[:, half:half + W]
                    if kc < 2:
                        nc.vector.tensor_copy(dst, ex[:, 0:W])
                    elif kc % 3 == 2:
                        nc.gpsimd.tensor_add(dst, dst, ex[:, 0:W])
                    else:
                        nc.vector.tensor_add(dst, dst, ex[:, 0:W])

                for r in range(N_CORES):
                    # stream rank r's K^T / V slabs for this head group
                    kt_slab = kvstream.tile([128, LKV], BF, tag="kt")
                    nc.sync.dma_start(
                        kt_slab[:],
                        gathK[g][r * 128 * LKV:(r + 1) * 128 * LKV]
                        .rearrange("(p c) -> p c", p=128))
                    v_slab = kvstream.tile([128, LKV // 128, D], BF, tag="v")
                    nc.sync.dma_start(
                        v_slab[:],
                        gathV[g][r * 128 * LKV:(r + 1) * 128 * LKV]
                        .rearrange("(a p d) -> p a d", p=128, d=D))
                    for l in range(LKV // 128):
                        kc = r * (LKV // 128) + l
                        st = stps.tile([128, W], F32, tag="st")
                        ex = attw.tile([128, W], BF, tag="ex")
                        if kc >= 56:
                            # strip-0 q cols are fully causal-masked for
                            # every core here: compute only the high half
                            # (the mask mul below zeroes the stale half)
                            for hh in range(4):
                                nc.tensor.matmul(
                                    st[:, hh * QS + 128:(hh + 1) * QS],
                                    kt_slab[:, l * 128:(l + 1) * 128],
                                    qt_sb[:, g * 4 + hh, 128:QS],
                                    start=True, stop=True)
                        else:
                            for hh in range(0, 4, 2):
                                nc.tensor.matmul(
                                    st[:, hh * QS:(hh + 2) * QS],
                                    kt_slab[:, l * 128:(l + 1) * 128],
                                    qt_sb[:, g * 4 + hh:g * 4 + hh + 2, :],
                                    start=True, stop=True)
                        if pend:
                            flush()
                        if kc >= 56:
                            nc.scalar.activation(
                                ex.rearrange(
                                    "p (h q) -> p h q", q=QS)[:, :, 128:],
                                st[:].rearrange(
                                    "p (h q) -> p h q", q=QS)[:, :, 128:],
                                mybir.ActivationFunctionType.Exp,
                                scale=SCALE)
                        else:
                            nc.scalar.activation(
                                ex[:], st[:],
                                mybir.ActivationFunctionType.Exp,
                                scale=SCALE)
                        if kc >= BND:
                            nc.vector.tensor_mul(
                                ex[:], ex[:], mask_sb[:, kc - BND, :])
                        pend.append((ex, l, v_slab, kc))
                flush()

                # copy attention output out unnormalized first, so ot_ps frees
                # for the next head group while the normalization tail runs
                nc.vector.tensor_copy(ao_sb[:, g, :], ot_ps[:])
                for nn in range(0, W, 512):
                    den = finps.tile([1, 512], F32, tag="fin")
                    nc.tensor.matmul(den[:], ones_k[:],
                                     acc2[:, nn:nn + 512],
                                     start=True, stop=False)
                    nc.tensor.matmul(den[:], ones_k[:],
                                     acc2[:, W + nn:W + nn + 512],
                                     start=False, stop=True)
                    recip_f = attw.tile([1, 512], F32, tag="recipf")
                    nc.vector.reciprocal(recip_f[:], den[:])
                    recip = attw.tile([1, 512], BF, tag="recip")
                    nc.vector.tensor_copy(recip[:], recip_f[:])
                    bc = finps.tile([128, 512], F32, tag="fin")
                    nc.tensor.matmul(bc[:], ones_1[:], recip[:],
                                     start=True, stop=True)
                    bc_sb = attw.tile([128, 512], BF, tag="bcsb")
                    nc.vector.tensor_copy(bc_sb[:], bc[:])
                    nc.vector.tensor_mul(ao_sb[:, g, nn:nn + 512],
                                         ao_sb[:, g, nn:nn + 512], bc_sb[:])

                # fold this head group into the output projection now (PE has
                # slack during the next group's attention); out_acc holds the
                # running f32 sum over groups
                wo_g = wop.tile([128, 4, HID], BF, tag="wog")
                nc.sync.dma_start(
                    wo_g[:],
                    woT.ap()[g * 512:(g + 1) * 512, :]
                    .rearrange("(a p) d -> p a d", p=128))
                for jc in range(HC):
                    fp = fps.tile([128, QS], F32, tag="fp")
                    for hh in range(4):
                        nc.tensor.matmul(
                            fp[:], wo_g[:, hh, jc * 128:(jc + 1) * 128],
                            ao_sb[:, g, hh * QS:(hh + 1) * QS],
                            start=(hh == 0), stop=(hh == 3))
                    if g == 0:
                        nc.vector.tensor_copy(out_acc[:, jc, :], fp[:])
                    else:
                        nc.vector.tensor_add(out_acc[:, jc, :],
                                             out_acc[:, jc, :], fp[:])

        # ---------------- Phase F: store the accumulated output -----------
        for jc in range(HC):
            nc.sync.dma_start(outT.ap()[jc * 128:(jc + 1) * 128, :],
                              out_acc[:, jc, :])


def _get_nc():
    if "nc" not in _CACHE:
        _CACHE["nc"] = _build()
    return _CACHE["nc"]


def _make_in_maps(x_q, x_kv, Wq, Wk, Wv, Wo):
    xqT_full = np.ascontiguousarray(x_q[0].T)           # [HID, SL]
    xkvT_full = np.ascontiguousarray(x_kv[0].T)         # [HID, SKV]
    wqT = np.ascontiguousarray(Wq.T).astype(BF16)
    wkT = np.ascontiguousarray(Wk.T).astype(BF16)
    wvT = np.ascontiguousarray(Wv.T).astype(BF16)
    woT = np.ascontiguousarray(Wo.T).astype(BF16)

    in_maps = []
    for c in range(N_CORES):
        s0, s1 = c, 15 - c
        xqT = np.concatenate(
            [xqT_full[:, s0 * 128:(s0 + 1) * 128],
             xqT_full[:, s1 * 128:(s1 + 1) * 128]], axis=1).astype(BF16)
        xkvT = np.ascontiguousarray(
            xkvT_full[:, c * LKV:(c + 1) * LKV]).astype(BF16)
        # causal masks for key chunks 48..63, replicated across the 4 heads
        # of a kv group (so one tensor_mul covers [128, 4*QS])
        mask = np.zeros((16, 128, QS), dtype=np.float32)
        kk = np.arange(128)
        for j in range(16):
            key_g = (BND + j) * 128 + kk                # [128]
            for half, st in enumerate((s0, s1)):
                q_g = RANK_OFF + st * 128 + np.arange(128)   # [128]
                mask[j, :, half * 128:(half + 1) * 128] = (
                    key_g[:, None] <= q_g[None, :])
        mask4 = np.tile(mask, (1, 1, 4))                # [16, 128, 4*QS]
        in_maps.append({
            "xqT": xqT, "xkvT": xkvT, "wqT": wqT, "wkT": wkT,
            "wvT": wvT, "woT": woT,
            "mask": mask4.reshape(16 * 128, 4 * QS).astype(BF16),
        })
    return in_maps


def _unshard(results):
    out = np.empty((1, SL, HID), dtype=np.float32)
    for c in range(N_CORES):
        outT = results[c]["outT"]                       # [HID, QS]
        s0, s1 = c, 15 - c
        out[0, s0 * 128:(s0 + 1) * 128, :] = outT[:, 0:128].T
        out[0, s1 * 128:(s1 + 1) * 128, :] = outT[:, 128:256].T
    return out


def kernel(x_q, x_kv, Wq, Wk, Wv, Wo, _trace=False, _result_box=None):
    nc = _get_nc()
    in_maps = _make_in_maps(x_q, x_kv, Wq, Wk, Wv, Wo)
    res = bass_utils.run_bass_kernel_spmd(
        nc, in_maps, core_ids=list(range(N_CORES)), trace=_trace)
    if _result_box is not None:
        _result_box.append(res)
    return _unshard(res.results)
